# revision 2
# baseline (speedup 1.0000x reference)
import numpy as np

CENTER = 8
TEMP = 5.0
N_CORES = 8
DEVICE_TIMEOUT_S = 420.0

# Problem nn_AdaptivePool_38697655337319 (hardcoded shapes):
#   text_features  [A=256, D=512]
#   video_features [B=256, V=12, D=512]
#   W1 [128, 256], b1 [256], W2 [256, 1], b2 [1]  ->  out [A=256, B=256] f32
# Sharding: data-parallel over the text axis A across the 8 NeuronCores;
# video_features and the MLP weights are replicated. Each core computes a
# [A/8, B] logits tile; the full output is the concatenation over cores.
# The device path runs in a watchdog subprocess so a wedged device stack
# can never hang the caller; on any failure we fall back to an exact
# numpy implementation.


def _shard_compute(text_sh, video, W1, b1, W2, b2):
    """Per-core computation on an A-shard. jax-traceable."""
    import jax
    import jax.numpy as jnp

    A, D = text_sh.shape
    B, V, _ = video.shape
    C = CENTER
    Wd = D // C

    # cross-modal attention pooling over frames
    v_weight = jnp.einsum('ad,bvd->abv', text_sh, video)
    v_weight = jax.nn.softmax(v_weight / TEMP, axis=-1)
    v_feat = jnp.einsum('abv,bvd->abd', v_weight, video)          # [a,B,D]

    t_feat = text_sh.reshape(A, C, Wd)                            # [a,C,W]
    v_feat = v_feat.reshape(A, B, C, Wd)                          # [a,B,C,W]

    # gating MLP on concat(text, video) chunks, concat avoided by
    # splitting W1 into its text / video row blocks
    W1t = W1[:Wd]
    W1v = W1[Wd:]
    t_part = jnp.einsum('acw,wh->ach', t_feat, W1t)               # [a,C,H]
    v_part = jnp.einsum('abcw,wh->abch', v_feat, W1v)             # [a,B,C,H]
    h = jax.nn.relu(t_part[:, None] + v_part + b1)
    weight = (jnp.einsum('abch,ho->abco', h, W2) + b2)[..., 0]    # [a,B,C]

    # L2-normalized per-center cosine similarity, gated sum
    _t = t_feat / jnp.linalg.norm(t_feat, axis=-1, keepdims=True)
    _v = v_feat / jnp.linalg.norm(v_feat, axis=-1, keepdims=True)
    logits = jnp.einsum('acd,abcd->abc', _t, _v)
    return jnp.einsum('abc,abc->ab', logits, weight)


def _device_child(conn, text_features, video_features, W1, b1, W2, b2):
    try:
        import jax

        devs = jax.devices()
        if len(devs) < N_CORES:
            raise RuntimeError(f"need {N_CORES} devices, have {len(devs)}")
        A = text_features.shape[0]
        text_sh = text_features.reshape(N_CORES, A // N_CORES, -1)
        fn = jax.pmap(
            _shard_compute,
            in_axes=(0, None, None, None, None, None),
            devices=devs[:N_CORES],
        )
        out = fn(text_sh, video_features, W1, b1, W2, b2)
        out = np.asarray(out).reshape(A, -1).astype(np.float32)
        if not np.all(np.isfinite(out)):
            raise RuntimeError("non-finite output from device path")
        conn.send(("ok", out))
    except Exception as e:  # noqa: BLE001
        try:
            conn.send(("err", repr(e)))
        except Exception:
            pass
    finally:
        conn.close()


def _kernel_device(text_features, video_features, W1, b1, W2, b2):
    import multiprocessing as mp

    ctx = mp.get_context("fork")
    parent, child = ctx.Pipe(duplex=False)
    p = ctx.Process(
        target=_device_child,
        args=(child, text_features, video_features, W1, b1, W2, b2),
    )
    p.start()
    child.close()
    result = None
    if parent.poll(DEVICE_TIMEOUT_S):
        try:
            result = parent.recv()
        except EOFError:
            result = None
    p.join(timeout=10.0)
    if p.is_alive():
        p.kill()
        p.join(timeout=10.0)
    if result is not None and result[0] == "ok":
        return result[1]
    raise RuntimeError(f"device path failed: {result!r}")


def _kernel_numpy(text_features, video_features, W1, b1, W2, b2):
    """Exact numpy implementation (fallback)."""
    A, D = text_features.shape
    B, V, _ = video_features.shape
    C = CENTER
    Wd = D // C
    t = text_features
    vid = video_features

    vw = np.einsum('ad,bvd->abv', t, vid) / TEMP
    vw = vw - vw.max(axis=-1, keepdims=True)
    np.exp(vw, out=vw)
    vw /= vw.sum(axis=-1, keepdims=True)
    v_feat = np.einsum('abv,bvd->abd', vw, vid).reshape(A, B, C, Wd)
    t_feat = t.reshape(A, C, Wd)

    W1t, W1v = W1[:Wd], W1[Wd:]
    t_part = np.einsum('acw,wh->ach', t_feat, W1t)
    weight = np.empty((A, B, C), dtype=np.float32)
    blk = 32  # block over A to bound the [a,B,C,H] intermediate
    for a0 in range(0, A, blk):
        v_part = np.einsum('abcw,wh->abch', v_feat[a0:a0 + blk], W1v)
        h = v_part + t_part[a0:a0 + blk, None] + b1
        np.maximum(h, 0.0, out=h)
        weight[a0:a0 + blk] = np.einsum('abch,ho->abc', h, W2) + b2

    _t = t_feat / np.linalg.norm(t_feat, axis=-1, keepdims=True)
    _v = v_feat / np.linalg.norm(v_feat, axis=-1, keepdims=True)
    logits = np.einsum('acd,abcd->abc', _t, _v)
    return np.einsum('abc,abc->ab', logits, weight).astype(np.float32)


def kernel(text_features, video_features, W1, b1, W2, b2):
    text_features = np.ascontiguousarray(text_features, dtype=np.float32)
    video_features = np.ascontiguousarray(video_features, dtype=np.float32)
    W1 = np.ascontiguousarray(W1, dtype=np.float32)
    b1 = np.ascontiguousarray(b1, dtype=np.float32)
    W2 = np.ascontiguousarray(W2, dtype=np.float32)
    b2 = np.ascontiguousarray(b2, dtype=np.float32)
    try:
        return _kernel_device(text_features, video_features, W1, b1, W2, b2)
    except Exception:
        return _kernel_numpy(text_features, video_features, W1, b1, W2, b2)


# revision 3
# speedup vs baseline: 1.7016x; 1.7016x over previous
import numpy as np

CENTER = 8
TEMP = 5.0
N_CORES = 8
DEVICE_TIMEOUT_S = 420.0

# Problem nn_AdaptivePool_38697655337319 (hardcoded shapes):
#   text_features  [A=256, D=512]
#   video_features [B=256, V=12, D=512]
#   W1 [128, 256], b1 [256], W2 [256, 1], b2 [1]  ->  out [A=256, B=256] f32
# Sharding: data-parallel over the text axis A across the 8 NeuronCores;
# video_features and the MLP weights are replicated. Each core computes a
# [A/8, B] logits tile; the full output is the concatenation over cores.
# The device path runs in a watchdog subprocess so a wedged device stack
# can never hang the caller; on any failure we fall back to an exact
# numpy implementation.


def _shard_compute(text_sh, video, W1, b1, W2, b2):
    """Per-core computation on an A-shard. jax-traceable."""
    import jax
    import jax.numpy as jnp

    A, D = text_sh.shape
    B, V, _ = video.shape
    C = CENTER
    Wd = D // C

    # cross-modal attention pooling over frames
    v_weight = jnp.einsum('ad,bvd->abv', text_sh, video)
    v_weight = jax.nn.softmax(v_weight / TEMP, axis=-1)
    v_feat = jnp.einsum('abv,bvd->abd', v_weight, video)          # [a,B,D]

    t_feat = text_sh.reshape(A, C, Wd)                            # [a,C,W]
    v_feat = v_feat.reshape(A, B, C, Wd)                          # [a,B,C,W]

    # gating MLP on concat(text, video) chunks, concat avoided by
    # splitting W1 into its text / video row blocks
    W1t = W1[:Wd]
    W1v = W1[Wd:]
    t_part = jnp.einsum('acw,wh->ach', t_feat, W1t)               # [a,C,H]
    v_part = jnp.einsum('abcw,wh->abch', v_feat, W1v)             # [a,B,C,H]
    h = jax.nn.relu(t_part[:, None] + v_part + b1)
    weight = (jnp.einsum('abch,ho->abco', h, W2) + b2)[..., 0]    # [a,B,C]

    # L2-normalized per-center cosine similarity, gated sum
    _t = t_feat / jnp.linalg.norm(t_feat, axis=-1, keepdims=True)
    _v = v_feat / jnp.linalg.norm(v_feat, axis=-1, keepdims=True)
    logits = jnp.einsum('acd,abcd->abc', _t, _v)
    return jnp.einsum('abc,abc->ab', logits, weight)


def _device_worker(conn):
    """Long-lived worker: init jax once, serve kernel requests over a pipe."""
    try:
        import jax

        devs = jax.devices()
        if len(devs) < N_CORES:
            raise RuntimeError(f"need {N_CORES} devices, have {len(devs)}")
        fn = jax.pmap(
            _shard_compute,
            in_axes=(0, None, None, None, None, None),
            devices=devs[:N_CORES],
        )
        conn.send(("ready", None))
    except Exception as e:  # noqa: BLE001
        try:
            conn.send(("err", repr(e)))
        except Exception:
            pass
        return
    while True:
        try:
            msg = conn.recv()
        except EOFError:
            return
        if msg is None:
            return
        try:
            text_features, video_features, W1, b1, W2, b2 = msg
            A = text_features.shape[0]
            text_sh = text_features.reshape(N_CORES, A // N_CORES, -1)
            out = fn(text_sh, video_features, W1, b1, W2, b2)
            out = np.asarray(out).reshape(A, -1).astype(np.float32)
            if not np.all(np.isfinite(out)):
                raise RuntimeError("non-finite output from device path")
            conn.send(("ok", out))
        except Exception as e:  # noqa: BLE001
            try:
                conn.send(("err", repr(e)))
            except Exception:
                return


_worker = None  # (process, parent_conn)


def _get_worker():
    global _worker
    if _worker is not None and _worker[0].is_alive():
        return _worker[1]
    import multiprocessing as mp

    ctx = mp.get_context("fork")
    parent, child = ctx.Pipe(duplex=True)
    p = ctx.Process(target=_device_worker, args=(child,), daemon=True)
    p.start()
    child.close()
    if not parent.poll(DEVICE_TIMEOUT_S):
        p.kill()
        raise RuntimeError("device worker init timeout")
    status, payload = parent.recv()
    if status != "ready":
        p.join(timeout=5.0)
        raise RuntimeError(f"device worker init failed: {payload}")
    _worker = (p, parent)
    return parent


def _kill_worker():
    global _worker
    if _worker is not None:
        try:
            _worker[0].kill()
        except Exception:
            pass
        _worker = None


def _kernel_device(text_features, video_features, W1, b1, W2, b2):
    conn = _get_worker()
    conn.send((text_features, video_features, W1, b1, W2, b2))
    if not conn.poll(DEVICE_TIMEOUT_S):
        _kill_worker()
        raise RuntimeError("device request timeout")
    status, payload = conn.recv()
    if status != "ok":
        _kill_worker()
        raise RuntimeError(f"device path failed: {payload}")
    return payload


def _kernel_numpy(text_features, video_features, W1, b1, W2, b2):
    """Exact numpy implementation (fallback)."""
    A, D = text_features.shape
    B, V, _ = video_features.shape
    C = CENTER
    Wd = D // C
    t = text_features
    vid = video_features

    vw = np.einsum('ad,bvd->abv', t, vid) / TEMP
    vw = vw - vw.max(axis=-1, keepdims=True)
    np.exp(vw, out=vw)
    vw /= vw.sum(axis=-1, keepdims=True)
    v_feat = np.einsum('abv,bvd->abd', vw, vid).reshape(A, B, C, Wd)
    t_feat = t.reshape(A, C, Wd)

    W1t, W1v = W1[:Wd], W1[Wd:]
    t_part = np.einsum('acw,wh->ach', t_feat, W1t)
    weight = np.empty((A, B, C), dtype=np.float32)
    blk = 32  # block over A to bound the [a,B,C,H] intermediate
    for a0 in range(0, A, blk):
        v_part = np.einsum('abcw,wh->abch', v_feat[a0:a0 + blk], W1v)
        h = v_part + t_part[a0:a0 + blk, None] + b1
        np.maximum(h, 0.0, out=h)
        weight[a0:a0 + blk] = np.einsum('abch,ho->abc', h, W2) + b2

    _t = t_feat / np.linalg.norm(t_feat, axis=-1, keepdims=True)
    _v = v_feat / np.linalg.norm(v_feat, axis=-1, keepdims=True)
    logits = np.einsum('acd,abcd->abc', _t, _v)
    return np.einsum('abc,abc->ab', logits, weight).astype(np.float32)


def kernel(text_features, video_features, W1, b1, W2, b2):
    text_features = np.ascontiguousarray(text_features, dtype=np.float32)
    video_features = np.ascontiguousarray(video_features, dtype=np.float32)
    W1 = np.ascontiguousarray(W1, dtype=np.float32)
    b1 = np.ascontiguousarray(b1, dtype=np.float32)
    W2 = np.ascontiguousarray(W2, dtype=np.float32)
    b2 = np.ascontiguousarray(b2, dtype=np.float32)
    try:
        return _kernel_device(text_features, video_features, W1, b1, W2, b2)
    except Exception:
        return _kernel_numpy(text_features, video_features, W1, b1, W2, b2)
